# revision 25
# baseline (speedup 1.0000x reference)
"""Bahdanau attention on 8 TRN2 NeuronCores, data-parallel over batch.

Math (per batch b):
    h1[s,u]  = sum_e v[s,e] * W1[u,e]
    t[s,u]   = tanh(h1[s,u] + Z[b,u])          Z = q@W2.T + W2_b + W1_b  (host)
    score[s] = sum_u V[u] * t[s,u]             (+V_b dropped: softmax shift-inv)
    attn     = softmax(score)                  (host, from device scores)
    ctx[e]   = sum_s exp(score[s]) * v[s,e] / sum_s exp(score[s])
               (device computes the unnormalized sum; host divides)

Raw bass (no Tile): the xbar transpose DMA (XPOSE) ISA slot carries at most
ONE sync wait, so Tile's auto-semaphores (lane-predecessor wait + WAR wait)
can never schedule it in a steady-state pipeline.  With manual semaphores the
waits become separate SP-sequencer instructions and the XPOSE itself carries
only its completion increment.

All DRAM->SBUF traffic is XPOSE; host pre-arranges every input so its
transpose lands in the exact SBUF layout the PE wants:

  xpose semantics: out[i,j,k] = in2d[k, j*a + i]   (out dims [a,b,c], a=parts)
  vt[p, jE, s]   = v[s, jE*128+p]   <- in2d = vals_s[b]  [2048(s), 1024(e)]
  stage[p, t, e] = v[t*128+p, e]    <- in2d = vals_e[b]  [1024(e), 2048(s)]
  w1_sb[p,jE,u]  = W1[u, jE*128+p]  <- in2d = W1_w       [512(u), 1024(e)] bf16
  vv_sb[p,0,k]   = Vpad[k, p]       <- in2d = Vpad       [16, 128] bf16
  zb_sb[p,ut,k]  = Zpad[k, ut*128+p]<- in2d = Zpad       [16, 512] fp16

PSUM budget (8 banks): h1 groups rotate over banks 0-3, score columns over
banks 4-5 (one per batch parity), context over banks 6-7 (one per e-half).

Pipeline is 2-deep over batches (vt/stage/tt/esc double buffered).
"""

import numpy as np
import ml_dtypes

import concourse.bass as bass
import concourse.mybir as mybir
from concourse.bass import ts, ds
from concourse.bass_utils import run_bass_kernel_spmd

F32 = mybir.dt.float32
BF16 = mybir.dt.bfloat16
FP16 = mybir.dt.float16
AFT = mybir.ActivationFunctionType

N_CORES = 8
BATCH = 64
B_PER_CORE = BATCH // N_CORES  # 8
SEQ = 2048
E = 1024   # 2u
U = 512
NT = SEQ // 128    # 16 seq chunks of 128
NEB = E // 128     # 8 e-blocks
NUT = U // 128     # 4 u-tiles
NG = NUT * (NT // 4)  # 16 h1 psum groups per batch


def build_nc():
    nc = bass.Bass()
    vals_s = nc.dram_tensor("vals_s", [B_PER_CORE, SEQ, E], BF16, kind="ExternalInput")
    vals_e = nc.dram_tensor("vals_e", [B_PER_CORE, 2, E, SEQ // 2], BF16, kind="ExternalInput")
    w1 = nc.dram_tensor("w1", [U, E], BF16, kind="ExternalInput")
    vpad = nc.dram_tensor("vpad", [16, 128], BF16, kind="ExternalInput")
    zpad = nc.dram_tensor("zpad", [16, U], FP16, kind="ExternalInput")
    sco = nc.dram_tensor("scores", [128, B_PER_CORE, NT], F32, kind="ExternalOutput")
    ctxo = nc.dram_tensor("ctx", [B_PER_CORE, E], F32, kind="ExternalOutput")

    B = B_PER_CORE
    from contextlib import ExitStack
    with ExitStack() as stack:
        w1_sb = stack.enter_context(nc.sbuf_tensor([128, NEB, U], BF16))
        vv_sb = stack.enter_context(nc.sbuf_tensor([128, 1, 16], BF16))
        zb_sb = stack.enter_context(nc.sbuf_tensor([128, NUT, 16], FP16))
        vt_sb = stack.enter_context(nc.sbuf_tensor([128, 4, NEB, SEQ // 2], BF16))
        st_sb = stack.enter_context(nc.sbuf_tensor([128, 4, NT // 2, E], BF16))
        tt_sb = stack.enter_context(nc.sbuf_tensor([128, 2, NG, 512], BF16))
        esc_sb = stack.enter_context(nc.sbuf_tensor([128, 2, NT], BF16))
        sca_sb = stack.enter_context(nc.sbuf_tensor([128, B, NT], F32))
        ctxa_sb = stack.enter_context(nc.sbuf_tensor([1, B, E], F32))
        h1_ps = stack.enter_context(nc.psum_tensor([128, 4, 512], F32))
        sc_ps = stack.enter_context(nc.psum_tensor([128, 2, 512], F32))
        cx_ps = stack.enter_context(nc.psum_tensor([128, 2, 512], F32))
        sem_names = ["S_w", "S_vt", "S_st", "S_h1g", "S_h1d", "S_scd",
                     "S_cxd", "S_tanh", "S_exp", "S_scc", "S_cxc", "S_out",
                     "S_sthf"]
        (S_w, S_vt, S_st, S_h1g, S_h1d, S_scd, S_cxd,
         S_tanh, S_exp, S_scc, S_cxc, S_out, S_sthf) = (
            stack.enter_context(nc.semaphore(name=n)) for n in sem_names
        )
        block = stack.enter_context(nc.Block())
        @block.sync
        def _(sync):
            sync.dma_start_transpose(w1_sb[:], w1[:]).then_inc(S_w, 16)
            sync.dma_start_transpose(vv_sb[:], vpad[:]).then_inc(S_w, 16)
            sync.dma_start_transpose(zb_sb[:], zpad[:]).then_inc(S_w, 16)
            for b in range(B):
                for h in range(2):
                    if b >= 2:
                        # half-slot free once the last h1 group of b-2
                        # reading s-half h (ut=3, tq=2h+1 -> g=13+2h) ran
                        sync.wait_ge(S_h1g, (b - 2) * NG + 14 + 2 * h)
                    sync.dma_start_transpose(
                        vt_sb[:, (b % 2) * 2 + h],
                        vals_s[b, ds(h * (SEQ // 2), SEQ // 2), :],
                    ).then_inc(S_vt, 16)
                for h in range(2):
                    if b >= 2:
                        if h == 0:
                            # half-0 slot free once ctx(b-2) read its half-0
                            sync.wait_ge(S_sthf, b - 1)
                        else:
                            # half-1 slot free once ctx(b-2) fully done
                            sync.wait_ge(S_cxd, 2 * (b - 2) + 2)
                    sync.dma_start_transpose(
                        st_sb[:, (b % 2) * 2 + h], vals_e[b, h]
                    ).then_inc(S_st, 16)
            # outputs; explicit XPOSE-complete waits guard the xbar-mode
            # transition (transpose ‖ copy is a known HW hang)
            sync.wait_ge(S_vt, 32 * B)
            sync.wait_ge(S_st, 32 * B)
            sync.wait_ge(S_scc, B)
            sync.dma_start(sco[:], sca_sb[:]).then_inc(S_out, 16)
            sync.wait_ge(S_cxc, 2 * B)
            sync.dma_start(ctxo[:], ctxa_sb[:]).then_inc(S_out, 16)

        def emit_ctx(tensor, cb):
            # context for batch cb (deferred one batch so exp(cb) is ready)
            sl = cb % 2
            tensor.wait_ge(S_exp, cb + 1)
            for h in range(2):
                tensor.wait_ge(S_st, 16 * (2 * cb + h + 1))
                for eh in range(2):
                    if cb >= 1 and h == 0:
                        # cx bank eh: DVE copy of (cb-1, eh) must be done
                        tensor.wait_ge(S_cxc, 2 * (cb - 1) + eh + 1)
                    for tl in range(NT // 2):
                        mm = tensor.matmul(
                            cx_ps[:1, eh],
                            lhsT=esc_sb[:, sl, ds(h * (NT // 2) + tl, 1)],
                            rhs=st_sb[:, sl * 2 + h, tl, ds(eh * 512, 512)],
                            start=(h == 0 and tl == 0),
                            stop=(h == 1 and tl == NT // 2 - 1),
                        )
                    if h == 1:
                        mm.then_inc(S_cxd, 1)
                if h == 0:
                    # half-0 of this stage pair fully consumed
                    mm.then_inc(S_sthf, 1)

        @block.tensor
        def _(tensor):
            tensor.wait_ge(S_w, 16)
            for b in range(B):
                if b == 0:
                    pass
                sl = b % 2
                for g in range(NG):
                    ut, tq = g // (NT // 4), g % (NT // 4)
                    gg = b * NG + g
                    # s-half tq//2 of this batch's vt must have landed
                    tensor.wait_ge(S_vt, 16 * (2 * b + tq // 2 + 1))
                    if gg >= 4:
                        # bank g%4 free once tanh of group gg-4 read it
                        tensor.wait_ge(S_tanh, gg - 3)
                    for jE in range(NEB):
                        mm = tensor.matmul(
                            h1_ps[:, g % 4],
                            lhsT=w1_sb[:, jE, ts(ut, 128)],
                            rhs=vt_sb[:, sl * 2 + tq // 2, jE, ts(tq % 2, 512)],
                            start=(jE == 0),
                            stop=(jE == NEB - 1),
                        )
                    mm.then_inc(S_h1g, 1)
                if b == 0:
                    tensor.wait_ge(S_w, 32)  # vv_sb loaded
                if b >= 2:
                    # sc bank b%2: exp and DVE copy of b-2 must be done
                    tensor.wait_ge(S_exp, b - 1)
                    tensor.wait_ge(S_scc, b - 1)
                for t in range(NT):
                    tq, q = t // 4, t % 4
                    if q == 0:
                        # cols of chunk-group tq need tanh groups
                        # {tq, 4+tq, 8+tq, 12+tq}; last emitted is 12+tq
                        tensor.wait_ge(S_tanh, b * NG + 12 + tq + 1)
                    for ut in range(NUT):
                        g = ut * (NT // 4) + tq
                        mm = tensor.matmul(
                            sc_ps[:, b % 2, ds(t, 1)],
                            lhsT=tt_sb[:, sl, g, ts(q, 128)],
                            rhs=vv_sb[:, 0, ds(ut, 1)],
                            start=(ut == 0),
                            stop=(ut == NUT - 1),
                        )
                mm.then_inc(S_scd, 1)
                if b >= 1:
                    emit_ctx(tensor, b - 1)
            emit_ctx(tensor, B - 1)

        @block.scalar
        def _(scalar):
            scalar.wait_ge(S_w, 48)
            for b in range(B):
                sl = b % 2
                for g in range(NG):
                    ut = g // (NT // 4)
                    gg = b * NG + g
                    scalar.wait_ge(S_h1g, gg + 1)
                    if b >= 2 and g == 0:
                        # tt slot b%2 free once score MMs of b-2 read it
                        scalar.wait_ge(S_scd, b - 1)
                    scalar.activation(
                        tt_sb[:, sl, g, :], h1_ps[:, g % 4], AFT.Tanh,
                        bias=zb_sb[:, ut, ds(b, 1)],
                    ).then_inc(S_tanh, 1)
                scalar.wait_ge(S_scd, b + 1)
                if b >= 2:
                    # esc slot b%2 free once ctx of b-2 read it
                    scalar.wait_ge(S_cxd, 2 * (b - 2) + 2)
                scalar.activation(
                    esc_sb[:, sl, :], sc_ps[:, b % 2, ds(0, NT)], AFT.Exp
                ).then_inc(S_exp, 1)

        @block.vector
        def _(vector):
            for b in range(B):
                # after exp(b): ACT and DVE must not read the same PSUM bank
                # concurrently
                vector.wait_ge(S_exp, b + 1)
                vector.tensor_copy(
                    sca_sb[:, b, :], sc_ps[:, b % 2, ds(0, NT)]
                ).then_inc(S_scc, 1)
                for eh in range(2):
                    vector.wait_ge(S_cxd, 2 * b + eh + 1)
                    vector.tensor_copy(
                        ctxa_sb[:, b, ds(eh * 512, 512)], cx_ps[:1, eh]
                    ).then_inc(S_cxc, 1)
    return nc


_NC_CACHE = None


def _get_nc():
    global _NC_CACHE
    if _NC_CACHE is None:
        _NC_CACHE = build_nc()
    return _NC_CACHE


def _run(in_maps, trace=False, **kw):
    nc = _get_nc()
    return run_bass_kernel_spmd(nc, in_maps, core_ids=list(range(N_CORES)),
                                trace=trace, **kw)


def kernel_full(query, values, W1_w, W1_b, W2_w, W2_b, V_w, V_b, trace=False, **kw):
    query = np.asarray(query, np.float32)
    values = np.asarray(values, np.float32)
    W1_w = np.asarray(W1_w, np.float32)
    W1_b = np.asarray(W1_b, np.float32)
    W2_w = np.asarray(W2_w, np.float32)
    W2_b = np.asarray(W2_b, np.float32)
    V_w = np.asarray(V_w, np.float32)
    V_b = np.asarray(V_b, np.float32)

    # host-side prep (tiny except the bf16 cast/layouts of values)
    Z = query @ W2_w.T + W2_b + W1_b                       # [64, 512]
    w1_bf = W1_w.astype(ml_dtypes.bfloat16)
    vpad = np.zeros((16, 128), ml_dtypes.bfloat16)
    vpad[:NUT] = V_w.reshape(NUT, 128).astype(ml_dtypes.bfloat16)
    vb = values.astype(ml_dtypes.bfloat16)                 # [2048, 64, 1024]

    in_maps = []
    for k in range(N_CORES):
        bs = slice(k * B_PER_CORE, (k + 1) * B_PER_CORE)
        sl = vb[:, bs, :]
        zpad = np.zeros((16, U), np.float16)
        zpad[:B_PER_CORE] = Z[bs].astype(np.float16)
        in_maps.append({
            "vals_s": np.ascontiguousarray(sl.transpose(1, 0, 2)),
            "vals_e": np.ascontiguousarray(
                sl.transpose(1, 2, 0).reshape(B_PER_CORE, E, 2, SEQ // 2)
                .transpose(0, 2, 1, 3)),
            "w1": w1_bf,
            "vpad": vpad,
            "zpad": zpad,
        })

    res = _run(in_maps, trace=trace, **kw)

    scores = np.empty((BATCH, SEQ), np.float32)
    ctx_raw = np.empty((BATCH, E), np.float32)
    for k, r in enumerate(res.results):
        bs = slice(k * B_PER_CORE, (k + 1) * B_PER_CORE)
        # scores dram [128(p), 8(b), 16(t)] ; s = t*128 + p
        scores[bs] = r["scores"].transpose(1, 2, 0).reshape(B_PER_CORE, SEQ)
        ctx_raw[bs] = r["ctx"]

    m = scores.max(axis=1, keepdims=True)
    ex = np.exp(scores - m)
    attn = ex / ex.sum(axis=1, keepdims=True)
    # normalizer consistent with the device's bf16 exp weights
    ebf = np.exp(scores).astype(ml_dtypes.bfloat16).astype(np.float32)
    context = ctx_raw / ebf.sum(axis=1, keepdims=True)

    return context.astype(np.float32), attn.astype(np.float32)[:, :, None], res


def kernel(**inputs):
    context, attn, _ = kernel_full(**inputs)
    return context, attn


# revision 26
# speedup vs baseline: 1.0058x; 1.0058x over previous
"""Bahdanau attention on 8 TRN2 NeuronCores, data-parallel over batch.

Math (per batch b):
    h1[s,u]  = sum_e v[s,e] * W1[u,e]
    t[s,u]   = tanh(h1[s,u] + Z[b,u])          Z = q@W2.T + W2_b + W1_b  (host)
    score[s] = sum_u V[u] * t[s,u]             (+V_b dropped: softmax shift-inv)
    attn     = softmax(score)                  (host, from device scores)
    ctx[e]   = sum_s exp(score[s]) * v[s,e] / sum_s exp(score[s])
               (device computes the unnormalized sum; host divides)

Raw bass (no Tile): the xbar transpose DMA (XPOSE) ISA slot carries at most
ONE sync wait, so Tile's auto-semaphores (lane-predecessor wait + WAR wait)
can never schedule it in a steady-state pipeline.  With manual semaphores the
waits become separate SP-sequencer instructions and the XPOSE itself carries
only its completion increment.

All DRAM->SBUF traffic is XPOSE; host pre-arranges every input so its
transpose lands in the exact SBUF layout the PE wants:

  xpose semantics: out[i,j,k] = in2d[k, j*a + i]   (out dims [a,b,c], a=parts)
  vt[p, jE, s]   = v[s, jE*128+p]   <- in2d = vals_s[b]  [2048(s), 1024(e)]
  stage[p, t, e] = v[t*128+p, e]    <- in2d = vals_e[b]  [1024(e), 2048(s)]
  w1_sb[p,jE,u]  = W1[u, jE*128+p]  <- in2d = W1_w       [512(u), 1024(e)] bf16
  vv_sb[p,0,k]   = Vpad[k, p]       <- in2d = Vpad       [16, 128] bf16
  zb_sb[p,ut,k]  = Zpad[k, ut*128+p]<- in2d = Zpad       [16, 512] fp16

PSUM budget (8 banks): h1 groups rotate over banks 0-3, score columns over
banks 4-5 (one per batch parity), context over banks 6-7 (one per e-half).

Pipeline is 2-deep over batches (vt/stage/tt/esc double buffered).
"""

import numpy as np
import ml_dtypes

import concourse.bass as bass
import concourse.mybir as mybir
from concourse.bass import ts, ds
from concourse.bass_utils import run_bass_kernel_spmd

F32 = mybir.dt.float32
BF16 = mybir.dt.bfloat16
FP16 = mybir.dt.float16
AFT = mybir.ActivationFunctionType

N_CORES = 8
BATCH = 64
B_PER_CORE = BATCH // N_CORES  # 8
SEQ = 2048
E = 1024   # 2u
U = 512
NT = SEQ // 128    # 16 seq chunks of 128
NEB = E // 128     # 8 e-blocks
NUT = U // 128     # 4 u-tiles
NG = NUT * (NT // 4)  # 16 h1 psum groups per batch


def build_nc():
    nc = bass.Bass()
    vals_s = nc.dram_tensor("vals_s", [B_PER_CORE, SEQ, E], BF16, kind="ExternalInput")
    vals_e = nc.dram_tensor("vals_e", [B_PER_CORE, 2, E, SEQ // 2], BF16, kind="ExternalInput")
    w1 = nc.dram_tensor("w1", [U, E], BF16, kind="ExternalInput")
    vpad = nc.dram_tensor("vpad", [16, 128], BF16, kind="ExternalInput")
    zpad = nc.dram_tensor("zpad", [16, U], FP16, kind="ExternalInput")
    sco = nc.dram_tensor("scores", [128, B_PER_CORE, NT], F32, kind="ExternalOutput")
    ctxo = nc.dram_tensor("ctx", [B_PER_CORE, E], F32, kind="ExternalOutput")

    B = B_PER_CORE
    from contextlib import ExitStack
    with ExitStack() as stack:
        w1_sb = stack.enter_context(nc.sbuf_tensor([128, NEB, U], BF16))
        vv_sb = stack.enter_context(nc.sbuf_tensor([128, 1, 16], BF16))
        zb_sb = stack.enter_context(nc.sbuf_tensor([128, NUT, 16], FP16))
        vt_sb = stack.enter_context(nc.sbuf_tensor([128, 4, NEB, SEQ // 2], BF16))
        st_sb = stack.enter_context(nc.sbuf_tensor([128, 4, NT // 2, E], BF16))
        tt_sb = stack.enter_context(nc.sbuf_tensor([128, 2, NG, 512], BF16))
        esc_sb = stack.enter_context(nc.sbuf_tensor([128, 2, NT], BF16))
        sca_sb = stack.enter_context(nc.sbuf_tensor([128, B, NT], F32))
        ctxa_sb = stack.enter_context(nc.sbuf_tensor([1, B, E], F32))
        h1_ps = stack.enter_context(nc.psum_tensor([128, 4, 512], F32))
        sc_ps = stack.enter_context(nc.psum_tensor([128, 2, 512], F32))
        cx_ps = stack.enter_context(nc.psum_tensor([128, 2, 512], F32))
        sem_names = ["S_w", "S_vt", "S_st", "S_h1g", "S_h1d", "S_scd",
                     "S_cxd", "S_tanh", "S_exp", "S_scc", "S_cxc", "S_out",
                     "S_sthf"]
        (S_w, S_vt, S_st, S_h1g, S_h1d, S_scd, S_cxd,
         S_tanh, S_exp, S_scc, S_cxc, S_out, S_sthf) = (
            stack.enter_context(nc.semaphore(name=n)) for n in sem_names
        )
        block = stack.enter_context(nc.Block())
        @block.sync
        def _(sync):
            # critical-path loads first: PE's first matmul needs vt(0,h0)+w1
            sync.dma_start_transpose(
                vt_sb[:, 0], vals_s[0, ds(0, SEQ // 2), :]
            ).then_inc(S_vt, 16)
            sync.dma_start_transpose(w1_sb[:], w1[:]).then_inc(S_w, 16)
            sync.dma_start_transpose(
                vt_sb[:, 1], vals_s[0, ds(SEQ // 2, SEQ // 2), :]
            ).then_inc(S_vt, 16)
            sync.dma_start_transpose(vv_sb[:], vpad[:]).then_inc(S_w, 16)
            sync.dma_start_transpose(zb_sb[:], zpad[:]).then_inc(S_w, 16)
            for b in range(B):
                for h in range(2):
                    if b == 0:
                        continue  # batch-0 vt halves issued above
                    if b >= 2:
                        # half-slot free once the last h1 group of b-2
                        # reading s-half h (ut=3, tq=2h+1 -> g=13+2h) ran
                        sync.wait_ge(S_h1g, (b - 2) * NG + 14 + 2 * h)
                    sync.dma_start_transpose(
                        vt_sb[:, (b % 2) * 2 + h],
                        vals_s[b, ds(h * (SEQ // 2), SEQ // 2), :],
                    ).then_inc(S_vt, 16)
                for h in range(2):
                    if b >= 2:
                        if h == 0:
                            # half-0 slot free once ctx(b-2) read its half-0
                            sync.wait_ge(S_sthf, b - 1)
                        else:
                            # half-1 slot free once ctx(b-2) fully done
                            sync.wait_ge(S_cxd, 2 * (b - 2) + 2)
                    sync.dma_start_transpose(
                        st_sb[:, (b % 2) * 2 + h], vals_e[b, h]
                    ).then_inc(S_st, 16)
            # outputs; explicit XPOSE-complete waits guard the xbar-mode
            # transition (transpose ‖ copy is a known HW hang)
            sync.wait_ge(S_vt, 32 * B)
            sync.wait_ge(S_st, 32 * B)
            sync.wait_ge(S_scc, B)
            sync.dma_start(sco[:], sca_sb[:]).then_inc(S_out, 16)
            sync.wait_ge(S_cxc, 2 * B)
            sync.dma_start(ctxo[:], ctxa_sb[:]).then_inc(S_out, 16)

        def emit_ctx(tensor, cb):
            # context for batch cb (deferred one batch so exp(cb) is ready)
            sl = cb % 2
            tensor.wait_ge(S_exp, cb + 1)
            for h in range(2):
                tensor.wait_ge(S_st, 16 * (2 * cb + h + 1))
                for eh in range(2):
                    if cb >= 1 and h == 0:
                        # cx bank eh: DVE copy of (cb-1, eh) must be done
                        tensor.wait_ge(S_cxc, 2 * (cb - 1) + eh + 1)
                    for tl in range(NT // 2):
                        mm = tensor.matmul(
                            cx_ps[:1, eh],
                            lhsT=esc_sb[:, sl, ds(h * (NT // 2) + tl, 1)],
                            rhs=st_sb[:, sl * 2 + h, tl, ds(eh * 512, 512)],
                            start=(h == 0 and tl == 0),
                            stop=(h == 1 and tl == NT // 2 - 1),
                        )
                    if h == 1:
                        mm.then_inc(S_cxd, 1)
                if h == 0:
                    # half-0 of this stage pair fully consumed
                    mm.then_inc(S_sthf, 1)

        @block.tensor
        def _(tensor):
            tensor.wait_ge(S_w, 16)
            for b in range(B):
                if b == 0:
                    pass
                sl = b % 2
                for g in range(NG):
                    ut, tq = g // (NT // 4), g % (NT // 4)
                    gg = b * NG + g
                    # s-half tq//2 of this batch's vt must have landed
                    tensor.wait_ge(S_vt, 16 * (2 * b + tq // 2 + 1))
                    if gg >= 4:
                        # bank g%4 free once tanh of group gg-4 read it
                        tensor.wait_ge(S_tanh, gg - 3)
                    for jE in range(NEB):
                        mm = tensor.matmul(
                            h1_ps[:, g % 4],
                            lhsT=w1_sb[:, jE, ts(ut, 128)],
                            rhs=vt_sb[:, sl * 2 + tq // 2, jE, ts(tq % 2, 512)],
                            start=(jE == 0),
                            stop=(jE == NEB - 1),
                        )
                    mm.then_inc(S_h1g, 1)
                if b == 0:
                    tensor.wait_ge(S_w, 32)  # vv_sb loaded
                if b >= 2:
                    # sc bank b%2: exp and DVE copy of b-2 must be done
                    tensor.wait_ge(S_exp, b - 1)
                    tensor.wait_ge(S_scc, b - 1)
                for t in range(NT):
                    tq, q = t // 4, t % 4
                    if q == 0:
                        # cols of chunk-group tq need tanh groups
                        # {tq, 4+tq, 8+tq, 12+tq}; last emitted is 12+tq
                        tensor.wait_ge(S_tanh, b * NG + 12 + tq + 1)
                    for ut in range(NUT):
                        g = ut * (NT // 4) + tq
                        mm = tensor.matmul(
                            sc_ps[:, b % 2, ds(t, 1)],
                            lhsT=tt_sb[:, sl, g, ts(q, 128)],
                            rhs=vv_sb[:, 0, ds(ut, 1)],
                            start=(ut == 0),
                            stop=(ut == NUT - 1),
                        )
                mm.then_inc(S_scd, 1)
                if b >= 1:
                    emit_ctx(tensor, b - 1)
            emit_ctx(tensor, B - 1)

        @block.scalar
        def _(scalar):
            scalar.wait_ge(S_w, 48)
            for b in range(B):
                sl = b % 2
                for g in range(NG):
                    ut = g // (NT // 4)
                    gg = b * NG + g
                    scalar.wait_ge(S_h1g, gg + 1)
                    if b >= 2 and g == 0:
                        # tt slot b%2 free once score MMs of b-2 read it
                        scalar.wait_ge(S_scd, b - 1)
                    scalar.activation(
                        tt_sb[:, sl, g, :], h1_ps[:, g % 4], AFT.Tanh,
                        bias=zb_sb[:, ut, ds(b, 1)],
                    ).then_inc(S_tanh, 1)
                scalar.wait_ge(S_scd, b + 1)
                if b >= 2:
                    # esc slot b%2 free once ctx of b-2 read it
                    scalar.wait_ge(S_cxd, 2 * (b - 2) + 2)
                scalar.activation(
                    esc_sb[:, sl, :], sc_ps[:, b % 2, ds(0, NT)], AFT.Exp
                ).then_inc(S_exp, 1)

        @block.vector
        def _(vector):
            for b in range(B):
                # after exp(b): ACT and DVE must not read the same PSUM bank
                # concurrently
                vector.wait_ge(S_exp, b + 1)
                vector.tensor_copy(
                    sca_sb[:, b, :], sc_ps[:, b % 2, ds(0, NT)]
                ).then_inc(S_scc, 1)
                for eh in range(2):
                    vector.wait_ge(S_cxd, 2 * b + eh + 1)
                    vector.tensor_copy(
                        ctxa_sb[:, b, ds(eh * 512, 512)], cx_ps[:1, eh]
                    ).then_inc(S_cxc, 1)
    return nc


_NC_CACHE = None


def _get_nc():
    global _NC_CACHE
    if _NC_CACHE is None:
        _NC_CACHE = build_nc()
    return _NC_CACHE


def _run(in_maps, trace=False, **kw):
    nc = _get_nc()
    return run_bass_kernel_spmd(nc, in_maps, core_ids=list(range(N_CORES)),
                                trace=trace, **kw)


def kernel_full(query, values, W1_w, W1_b, W2_w, W2_b, V_w, V_b, trace=False, **kw):
    query = np.asarray(query, np.float32)
    values = np.asarray(values, np.float32)
    W1_w = np.asarray(W1_w, np.float32)
    W1_b = np.asarray(W1_b, np.float32)
    W2_w = np.asarray(W2_w, np.float32)
    W2_b = np.asarray(W2_b, np.float32)
    V_w = np.asarray(V_w, np.float32)
    V_b = np.asarray(V_b, np.float32)

    # host-side prep (tiny except the bf16 cast/layouts of values)
    Z = query @ W2_w.T + W2_b + W1_b                       # [64, 512]
    w1_bf = W1_w.astype(ml_dtypes.bfloat16)
    vpad = np.zeros((16, 128), ml_dtypes.bfloat16)
    vpad[:NUT] = V_w.reshape(NUT, 128).astype(ml_dtypes.bfloat16)
    vb = values.astype(ml_dtypes.bfloat16)                 # [2048, 64, 1024]

    in_maps = []
    for k in range(N_CORES):
        bs = slice(k * B_PER_CORE, (k + 1) * B_PER_CORE)
        sl = vb[:, bs, :]
        zpad = np.zeros((16, U), np.float16)
        zpad[:B_PER_CORE] = Z[bs].astype(np.float16)
        in_maps.append({
            "vals_s": np.ascontiguousarray(sl.transpose(1, 0, 2)),
            "vals_e": np.ascontiguousarray(
                sl.transpose(1, 2, 0).reshape(B_PER_CORE, E, 2, SEQ // 2)
                .transpose(0, 2, 1, 3)),
            "w1": w1_bf,
            "vpad": vpad,
            "zpad": zpad,
        })

    res = _run(in_maps, trace=trace, **kw)

    scores = np.empty((BATCH, SEQ), np.float32)
    ctx_raw = np.empty((BATCH, E), np.float32)
    for k, r in enumerate(res.results):
        bs = slice(k * B_PER_CORE, (k + 1) * B_PER_CORE)
        # scores dram [128(p), 8(b), 16(t)] ; s = t*128 + p
        scores[bs] = r["scores"].transpose(1, 2, 0).reshape(B_PER_CORE, SEQ)
        ctx_raw[bs] = r["ctx"]

    m = scores.max(axis=1, keepdims=True)
    ex = np.exp(scores - m)
    attn = ex / ex.sum(axis=1, keepdims=True)
    # normalizer consistent with the device's bf16 exp weights
    ebf = np.exp(scores).astype(ml_dtypes.bfloat16).astype(np.float32)
    context = ctx_raw / ebf.sum(axis=1, keepdims=True)

    return context.astype(np.float32), attn.astype(np.float32)[:, :, None], res


def kernel(**inputs):
    context, attn, _ = kernel_full(**inputs)
    return context, attn
